# revision 15
# baseline (speedup 1.0000x reference)
"""Trainium2 Bass kernel for nn_BiLSTM_50500225466406 — v3 (bf16 + col-group dirs).

2-layer BiLSTM (H=200) over word embeddings (E=300), B=32, S=128, + sigmoid
linear head (17 tags).  Char-CNN branch in the reference is dead code.

v3 strategy (vs fp32 baseline, which traced at PE 98.6% busy / 4 cyc/row):
  - all matmul operands bf16 (1 cyc/row)
  - direction d's gates at PSUM partitions 32d:32d+4 via PE col-group
    tile_position=(0, 32d): d0/d1 matmuls overlap in the array, and one
    ACT/DVE instruction covers both directions' elementwise work
    (rows 4:32 are dead lanes — lane-parallel engines charge free-dim only)
  - h accumulators (A tiles) hold time-ordered contiguous regions
    [fwd 0:4S | bwd 4S:8S] so every matmul AP is 1-D; time reversal is done
    by compile-time block indexing (bwd injection reads block S-1-s, tail
    copies write bwd h to its time slot)
  - c-state in SBUF; tanh(o) off the critical chain; split gate-tanh per
    PSUM bank so it starts as soon as bank0 closes; one STT of the B-term
    runs on GpSimd to unload DVE
  - gate math (tanh-only trick, doubled states):
      sigma(x) = (tanh(x/2)+1)/2 ; ct = 2c ; ht = 2h
      A = (tf+1)*ct ; B = (ti+1)*tg ; ct' = 0.5*A + B
      to' = tanh(o/2) ; tc = tanh(0.5*ct') ; ht = (to'+1)*tc
"""

import sys

for _p in ("/opt/trn_rl_repo",):
    if _p not in sys.path:
        sys.path.insert(0, _p)

import numpy as np
import ml_dtypes

import concourse.bass as bass
import concourse.mybir as mybir
import concourse.tile as tile
from concourse import bass_utils

F32 = mybir.dt.float32
BF16 = mybir.dt.bfloat16
AF = mybir.ActivationFunctionType
ALU = mybir.AluOpType
BF = ml_dtypes.bfloat16

# Problem constants (hardcoded per contract).
B, S, E, H, V, TAGS = 32, 128, 300, 200, 50002, 17
NCORES = 8
BL = B // NCORES          # 4 sentences per core
SB = S * BL               # 512 = time*batch columns per core
XK = (128, 128, 48)       # X^T partition chunks (300 emb dims + 1 ones + pad)
XROWS = 304
X1K = (128, 72, 128, 72, 1)   # layer-1 input chunks: fwd h(128,72), bwd h(128,72), ones
X1ROWS = 401
UCHUNKS = ((0, 128), (128, 128), (256, 128), (384, 16),
           (400, 128), (528, 128), (656, 128), (784, 16))  # 800 units, edge at 400


def _set_seq(s_len):
    """Dev helper: shrink the sequence length (and SB) for fast simulation."""
    global S, SB
    S = s_len
    SB = S * BL


def _prep_weights(emb_table, lstm_Wih0, lstm_Whh0, lstm_b0,
                  lstm_Wih1, lstm_Whh1, lstm_b1, out_w, out_b):
    """Host-side weight transforms shared by all cores (bf16 outputs)."""
    f32 = np.float32

    # layer0 input weights + bias: rows = 300 emb dims + ones row + pad
    w0 = np.zeros((XROWS, 1600), f32)
    for d in range(2):
        wt = lstm_Wih0[d].T.astype(f32).copy()        # [300, 800]
        wt[:, 0:400] *= 0.5                           # i,f halved
        wt[:, 600:800] *= 0.5                         # o halved too
        b = lstm_b0[d].astype(f32).copy()
        b[0:400] *= 0.5
        b[600:800] *= 0.5
        w0[0:300, d * 800:(d + 1) * 800] = wt
        w0[300, d * 800:(d + 1) * 800] = b

    # layer0 recurrent weights: input is ht=2h -> *0.5 ; i,f further *0.5
    u0 = np.zeros((200, 1600), f32)
    for d in range(2):
        ut = lstm_Whh0[d].T.astype(f32) * 0.5
        ut = ut.copy()
        ut[:, 0:400] *= 0.5
        ut[:, 600:800] *= 0.5
        u0[:, d * 800:(d + 1) * 800] = ut

    # layer1 input weights: input is ht-concat (=2x) -> *0.5 ; i,f *0.5 ; bias plain
    w1 = np.zeros((X1ROWS, 1600), f32)
    for d in range(2):
        wt = lstm_Wih1[d].T.astype(f32) * 0.5         # [400, 800]
        wt = wt.copy()
        wt[:, 0:400] *= 0.5
        wt[:, 600:800] *= 0.5
        b = lstm_b1[d].astype(f32).copy()
        b[0:400] *= 0.5
        b[600:800] *= 0.5
        w1[0:400, d * 800:(d + 1) * 800] = wt
        w1[400, d * 800:(d + 1) * 800] = b

    u1 = np.zeros((200, 1600), f32)
    for d in range(2):
        ut = lstm_Whh1[d].T.astype(f32) * 0.5
        ut = ut.copy()
        ut[:, 0:400] *= 0.5
        ut[:, 600:800] *= 0.5
        u1[:, d * 800:(d + 1) * 800] = ut

    # output head: input is ht-concat -> *0.5 ; bias row plain
    ow = np.zeros((X1ROWS, TAGS), f32)
    ow[0:400, :] = out_w.T.astype(f32) * 0.5
    ow[400, :] = out_b.astype(f32)

    i128 = np.eye(128, dtype=f32)
    return {k: v.astype(BF) for k, v in
            {"w0": w0, "u0": u0, "w1": w1, "u1": u1, "ow": ow,
             "i128": i128}.items()}


def _prep_xt(emb_table, words_shard):
    """[304, SB] X^T for one core: col 4t+b = emb[words[b, t]] (time order).

    row 300 = 1.0 (bias row), rows 301:304 = 0.
    """
    emb = np.asarray(emb_table)
    w = np.asarray(words_shard)                       # [BL, S]
    idx_f = w.T.reshape(-1)                           # n = t*BL + b
    xt = np.zeros((XROWS, SB), np.float32)
    xt[0:300, 0:SB] = emb[idx_f].T.astype(np.float32)
    xt[300, :] = 1.0
    return xt.astype(BF)


# --------------------------------------------------------------------------
# Bass program
# --------------------------------------------------------------------------

def _emit_xw_precompute(nc, tc, psum_pool, w_chunks, rhs_chunks, xw, tag):
    """xw[0:cw, ci*8S + 8t + 4d + b] = sum_k w_chunks[k][:, d*800+c0+..].T @ rhs

    rhs_chunks: list of rhs APs [Kp, SB], columns in time order 4t+b (shared
    by both directions).  Output copied PSUM->SBUF (bf16) into 8-wide
    time-ordered interleaved blocks; the injection matmuls pick block
    t=s (fwd) / t=S-1-s (bwd) at compile time.
    """
    nk = len(w_chunks)
    xwr = xw.rearrange("p (c s q b) -> p c s q b", c=8, s=S, q=2, b=BL)
    for d in range(2):
        for ci, (c0, cw) in enumerate(UCHUNKS):
            ps = psum_pool.tile([128, SB], F32, tag=tag, name=f"{tag}_ps")
            col = d * 800 + c0
            for k in range(nk):
                nc.tensor.matmul(
                    ps[0:cw, 0:SB],
                    w_chunks[k][:, col:col + cw],
                    rhs_chunks[k],
                    start=(k == 0), stop=(k == nk - 1),
                )
            src = ps[0:cw, 0:SB].rearrange("p (s b) -> p s b", s=S)
            dst = xwr[0:cw, ci, :, d, :]
            if (d * 8 + ci) % 2 == 0:
                nc.vector.tensor_copy(dst, src)
            else:
                nc.scalar.copy(dst, src)


def _emit_lstm_layer(nc, tc, xw, ua, ub, Aa, Ab, G, TT, TT2, Gw, Cst,
                     Ttile, Atmp, Btmp, TCa, Hh, Hsa, Hsb, I, s_len=None):
    """Emit one full BiLSTM layer (both directions, S steps unrolled).

    Direction d's gates live at PSUM partitions 32d:32d+4 (PE col-groups via
    tile_position=(0, 32d)).  G[i][bank] are per-bank ping-pong PSUM tiles so
    the gate tanh of bank0 starts as soon as bank0's matmuls close.
    Aa: [128, 8*S] bf16, Ab: [72, 8*S] bf16 — h accumulators, two
    time-ordered regions [fwd 0:4S | bwd 4S:8S].
    Gw is a scratch PSUM tile for PE warm-keeper matmuls (HAM K=8/8).
    """
    if s_len is None:
        s_len = S
    FS = 4 * s_len            # bwd region column offset in A tiles
    xwr = xw.rearrange("p (c s q) -> p c s q", c=8, s=s_len)
    # stale-PSUM guard: garbage rows 4:32 flow through tanh and the identity
    # transpose; Inf/NaN there would contaminate via 0*NaN in the PE sum.
    for i in range(2):
        nc.vector.memset(G[i][0][0:36, :], 0.0)
        nc.vector.memset(G[i][1][0:36, :], 0.0)
    nc.vector.memset(Cst[0:36, 0:H], 0.0)

    def inj(g, bank, s):
        for ci, (c0, cw) in enumerate(UCHUNKS):
            if (c0 < 400) != (bank == 0):
                continue
            for d in range(2):
                t = s if d == 0 else s_len - 1 - s
                nc.tensor.matmul(
                    g[32 * d:32 * d + 4, c0 - 400 * bank:c0 - 400 * bank + cw],
                    xwr[0:cw, ci, t, 4 * d:4 * d + 4],
                    I[0:cw, 0:cw],
                    start=(ci == 0 or ci == 4),
                    stop=(s == 0 and (ci == 3 or ci == 7)),
                    skip_group_check=True,
                    tile_position=(0, 32 * d),
                )

    def rec(g, bank, s):
        n0, n1 = (0, 400) if bank == 0 else (400, 800)
        for cj, (Ht, rows) in enumerate(((Hsa, 128), (Hsb, 72))):
            u = ua if rows == 128 else ub
            for d in range(2):
                # prev h of dir d from the step-indexed ring (written step s-1)
                nc.tensor.matmul(
                    g[32 * d:32 * d + 4, n0 - 400 * bank:n1 - 400 * bank],
                    Ht[(s - 1) % 2][0:rows, 4 * d:4 * d + 4],
                    u[0:rows, 800 * d + n0:800 * d + n1],
                    start=False, stop=(cj == 1),
                    skip_group_check=True,
                    tile_position=(0, 32 * d),
                )

    for s in range(s_len):
        g0, g1 = G[s % 2]
        Tt = Ttile[s % 2]
        TC = TCa[s % 2]
        Hc = Hh[s % 2]
        inj(g0, 0, s)
        if s > 0:
            rec(g0, 0, s)
        inj(g1, 1, s)
        if s > 0:
            rec(g1, 1, s)
        # ---- gate activations + cell update (rows 4:32 are dead lanes)
        nc.scalar.activation(Tt[0:36, 200:400], g0[0:36, 200:400], AF.Tanh)
        nc.scalar.activation(Tt[0:36, 0:200], g0[0:36, 0:200], AF.Tanh)
        nc.scalar.activation(Tt[0:36, 400:600], g1[0:36, 0:200], AF.Tanh)
        nc.scalar.activation(Tt[0:36, 600:800], g1[0:36, 200:400], AF.Tanh)
        nc.vector.scalar_tensor_tensor(
            Atmp[0:36, 0:H], Tt[0:36, 200:400], 1.0,
            Cst[0:36, 0:H], ALU.add, ALU.mult)
        nc.vector.scalar_tensor_tensor(
            Btmp[0:36, 0:H], Tt[0:36, 0:200], 1.0,
            Tt[0:36, 400:600], ALU.add, ALU.mult)
        nc.vector.scalar_tensor_tensor(
            Cst[0:36, 0:H], Atmp[0:36, 0:H], 0.5,
            Btmp[0:36, 0:H], ALU.mult, ALU.add)
        nc.scalar.activation(TC[0:36, 0:H], Cst[0:36, 0:H], AF.Tanh,
                             scale=0.5)
        # ---- ht = (to'+1)*tc, then transpose into the A buffers
        nc.vector.scalar_tensor_tensor(
            Hc[0:36, 0:H], Tt[0:36, 600:800], 1.0, TC[0:36, 0:H],
            ALU.add, ALU.mult)
        nc.tensor.matmul(TT[0:128, 0:36], Hc[0:36, 0:128], I[0:36, 0:36],
                         start=True, stop=True, skip_group_check=True)
        nc.tensor.matmul(TT2[0:72, 0:36], Hc[0:36, 128:200], I[0:36, 0:36],
                         start=True, stop=True, skip_group_check=True)
        # step-indexed h ring feeds the next step's recurrent matmuls
        ttr = TT.rearrange("p (j b) -> p j b", b=4)
        tt2r = TT2.rearrange("p (j b) -> p j b", b=4)
        nc.vector.tensor_copy(
            Hsa[s % 2][0:128, 0:8].rearrange("p (j b) -> p j b", b=4),
            ttr[0:128, 0::8, :])
        nc.vector.tensor_copy(
            Hsb[s % 2][0:72, 0:8].rearrange("p (j b) -> p j b", b=4),
            tt2r[0:72, 0::8, :])
        # fwd h_s -> time block s; bwd h (time s_len-1-s) -> its time block
        tb = FS + 4 * (s_len - 1 - s)
        nc.vector.tensor_copy(Aa[0:128, 4 * s:4 * s + 4], TT[0:128, 0:4])
        nc.vector.tensor_copy(Aa[0:128, tb:tb + 4], TT[0:128, 32:36])
        nc.scalar.copy(Ab[0:72, 4 * s:4 * s + 4], TT2[0:72, 0:4])
        nc.scalar.copy(Ab[0:72, tb:tb + 4], TT2[0:72, 32:36])


def _fix_pe_multiwaits(nc):
    """Walrus codegen rejects PE Matmult with >1 sync wait (LDWEIGHTS struct
    has a single wait slot).  Hoist extra waits onto PE NoOps inserted just
    before the offending matmult."""
    total = 0
    for fnc in nc.m.functions:
        for blk in fnc.blocks:
            lst = blk.instructions
            out = []
            for ins in lst:
                si = ins.sync_info
                cap = 1
                if si is not None and len(si.on_wait) > cap:
                    si_cls = type(si)
                    extra = list(si.on_wait[:-cap])
                    keep = si.on_wait[-cap]
                    for j, w in enumerate(extra):
                        nop = mybir.InstNoOp(
                            name=f"{ins.name}_wnop{j}", ins=[], outs=[])
                        nop.engine = ins.engine
                        nop.sync_info = si_cls(on_wait=[w], on_update=[])
                        out.append(nop)
                    ins.sync_info = si_cls(on_wait=[keep],
                                           on_update=list(si.on_update))
                    total += 1
                out.append(ins)
            blk.instructions = out
    return total


def build_program(fix_multiwait=True):
    nc = bass.Bass("TRN2", target_bir_lowering=False, debug=False)

    # ---- DRAM tensors (per-core inputs; SPMD same program)
    d_xt = nc.dram_tensor("xt", [XROWS, SB], BF16, kind="ExternalInput").ap()
    d_w0 = nc.dram_tensor("w0", [XROWS, 1600], BF16, kind="ExternalInput").ap()
    d_u0 = nc.dram_tensor("u0", [200, 1600], BF16, kind="ExternalInput").ap()
    d_w1 = nc.dram_tensor("w1", [X1ROWS, 1600], BF16, kind="ExternalInput").ap()
    d_u1 = nc.dram_tensor("u1", [200, 1600], BF16, kind="ExternalInput").ap()
    d_ow = nc.dram_tensor("ow", [X1ROWS, TAGS], BF16, kind="ExternalInput").ap()
    d_i128 = nc.dram_tensor("i128", [128, 128], BF16, kind="ExternalInput").ap()
    d_out = nc.dram_tensor("out", [BL, S, TAGS], F32, kind="ExternalOutput").ap()

    with tile.TileContext(nc) as tc:
        with tc.sbuf_pool(name="persist", bufs=1) as SP, \
             tc.psum_pool(name="gates", bufs=1) as GP:
            # persistent SBUF tiles
            I = SP.tile([128, 128], BF16, name="ident")
            u0a = SP.tile([128, 1600], BF16, name="u0a")
            u0b = SP.tile([72, 1600], BF16, name="u0b")
            u1a = SP.tile([128, 1600], BF16, name="u1a")
            u1b = SP.tile([72, 1600], BF16, name="u1b")
            xw0 = SP.tile([128, 8 * 8 * S], BF16, name="xw0")
            xw1 = SP.tile([128, 8 * 8 * S], BF16, name="xw1")
            A0a = SP.tile([128, 8 * S], BF16, name="A0a")
            A0b = SP.tile([72, 8 * S], BF16, name="A0b")
            A1a = SP.tile([128, 8 * S], BF16, name="A1a")
            A1b = SP.tile([72, 8 * S], BF16, name="A1b")
            ones = SP.tile([1, SB], BF16, name="ones")
            owc = [SP.tile([kk, TAGS], BF16, name=f"owc{k}")
                   for k, kk in enumerate(X1K)]
            # work tiles (shared by both layers)
            Cst = SP.tile([36, 256], BF16, name="Cst")
            Ttile = [SP.tile([36, 800], BF16, name=f"Ttile{i}") for i in range(2)]
            Atmp = SP.tile([36, 200], BF16, name="Atmp")
            Btmp = SP.tile([36, 200], BF16, name="Btmp")
            TCa = [SP.tile([36, 200], BF16, name=f"TCa{i}") for i in range(2)]
            Hh = [SP.tile([36, 200], BF16, name=f"Hh{i}") for i in range(2)]
            Hsa = [SP.tile([128, 8], BF16, name=f"Hsa{i}") for i in range(2)]
            Hsb = [SP.tile([72, 8], BF16, name=f"Hsb{i}") for i in range(2)]
            # PSUM: per-bank gate ping-pong + transpose + warm-keeper tiles
            G = [[GP.tile([36, 512], F32, name=f"G{i}b{b}") for b in range(2)]
                 for i in range(2)]
            TT = GP.tile([128, 36], F32, name="TT")
            TT2 = GP.tile([72, 36], F32, name="TT2")
            Gw = None

            # ---- load persistent weights
            nc.sync.dma_start(I, d_i128)
            nc.sync.dma_start(u0a, d_u0[0:128, :])
            nc.sync.dma_start(u0b, d_u0[128:200, :])
            nc.sync.dma_start(u1a, d_u1[0:128, :])
            nc.sync.dma_start(u1b, d_u1[128:200, :])
            nc.vector.memset(ones[0:1, 0:SB], 1.0)
            row = 0
            for k, kk in enumerate(X1K):
                nc.sync.dma_start(owc[k], d_ow[row:row + kk, :])
                row += kk

            # ---- phase 2: xw0 precompute
            with tc.sbuf_pool(name="ph2", bufs=1) as P2S, \
                 tc.psum_pool(name="ph2p", bufs=2) as P2P:
                xTc = [P2S.tile([XK[k], SB], BF16, name=f"xTc{k}")
                       for k in range(3)]
                w0c = [P2S.tile([XK[k], 1600], BF16, name=f"w0c{k}")
                       for k in range(3)]
                row = 0
                for k, kk in enumerate(XK):
                    nc.sync.dma_start(xTc[k], d_xt[row:row + kk, :])
                    nc.sync.dma_start(w0c[k], d_w0[row:row + kk, :])
                    row += kk
                _emit_xw_precompute(
                    nc, tc, P2P, w0c,
                    [xTc[k][:, 0:SB] for k in range(3)],
                    xw0, tag="xw0p")

            # ---- phase 3: layer-0 recurrence
            _emit_lstm_layer(nc, tc, xw0, u0a, u0b, A0a, A0b, G, TT, TT2,
                             Gw, Cst, Ttile, Atmp, Btmp, TCa, Hh, Hsa, Hsb, I)

            # ---- phase 4: xw1 precompute (input = A0 buffers + ones);
            # A regions are time-ordered, so rhs is shared by both dirs.
            rhs1 = [A0a[0:128, 0:4 * S], A0b[0:72, 0:4 * S],
                    A0a[0:128, 4 * S:8 * S], A0b[0:72, 4 * S:8 * S],
                    ones[0:1, 0:SB]]

            with tc.sbuf_pool(name="ph4", bufs=1) as P4S, \
                 tc.psum_pool(name="ph4p", bufs=2) as P4P:
                w1c = [P4S.tile([X1K[k], 1600], BF16, name=f"w1c{k}")
                       for k in range(5)]
                row = 0
                for k, kk in enumerate(X1K):
                    nc.sync.dma_start(w1c[k], d_w1[row:row + kk, :])
                    row += kk
                _emit_xw_precompute(nc, tc, P4P, w1c, rhs1, xw1, tag="xw1p")

            # ---- phase 5: layer-1 recurrence
            _emit_lstm_layer(nc, tc, xw1, u1a, u1b, A1a, A1b, G, TT, TT2,
                             Gw, Cst, Ttile, Atmp, Btmp, TCa, Hh, Hsa, Hsb, I)

            # ---- phase 6: output head (A1 regions are time-ordered)
            with tc.sbuf_pool(name="fin", bufs=2) as FS, \
                 tc.psum_pool(name="finp", bufs=2) as FP:
                out_r = d_out.rearrange("b t e -> t b e")
                ts = min(32, S)            # time steps per M-group
                mt = ts * BL
                for m in range(S // ts):
                    t0, t1 = m * ts, (m + 1) * ts
                    po = FP.tile([mt, TAGS], F32, tag="po", name="po")
                    lhs_chunks = [
                        A1a[0:128, 4 * t0:4 * t1],
                        A1b[0:72, 4 * t0:4 * t1],
                        A1a[0:128, 4 * S + 4 * t0:4 * S + 4 * t1],
                        A1b[0:72, 4 * S + 4 * t0:4 * S + 4 * t1],
                        ones[0:1, 0:mt],
                    ]
                    for k in range(5):
                        nc.tensor.matmul(
                            po[0:mt, 0:TAGS], lhs_chunks[k], owc[k],
                            start=(k == 0), stop=(k == 4),
                        )
                    so = FS.tile([mt, TAGS], F32, tag="so", name="so")
                    nc.scalar.activation(so[0:mt, 0:TAGS], po[0:mt, 0:TAGS],
                                         AF.Sigmoid)
                    nc.sync.dma_start(out_r[ts * m:ts * (m + 1), :, :],
                                      so[0:mt, 0:TAGS])

    if fix_multiwait:
        _fix_pe_multiwaits(nc)
    return nc


_CACHE = {}


def kernel(**inputs):
    inputs = {k: np.asarray(v) for k, v in inputs.items()}
    words = inputs["words"]

    shared = _prep_weights(
        inputs["emb_table"], inputs["lstm_Wih0"], inputs["lstm_Whh0"],
        inputs["lstm_b0"], inputs["lstm_Wih1"], inputs["lstm_Whh1"],
        inputs["lstm_b1"], inputs["out_w"], inputs["out_b"])

    in_maps = []
    for c in range(NCORES):
        xt = _prep_xt(inputs["emb_table"], words[c * BL:(c + 1) * BL])
        in_maps.append({"xt": xt, **shared})

    if "nc" not in _CACHE:
        _CACHE["nc"] = build_program()
    nc = _CACHE["nc"]

    res = bass_utils.run_bass_kernel_spmd(
        nc, in_maps, core_ids=list(range(NCORES)),
        trace=_CACHE.get("trace", False),
        tmpdir=_CACHE.get("tmpdir"))
    _CACHE["last_exec_ns"] = res.exec_time_ns
    _CACHE["last_res"] = res

    out = np.concatenate([res.results[c]["out"] for c in range(NCORES)], axis=0)
    return out.astype(np.float32)


# revision 16
# speedup vs baseline: 1.0227x; 1.0227x over previous
"""Trainium2 Bass kernel for nn_BiLSTM_50500225466406 — v3 (bf16 + col-group dirs).

2-layer BiLSTM (H=200) over word embeddings (E=300), B=32, S=128, + sigmoid
linear head (17 tags).  Char-CNN branch in the reference is dead code.

v3 strategy (vs fp32 baseline, which traced at PE 98.6% busy / 4 cyc/row):
  - all matmul operands bf16 (1 cyc/row)
  - direction d's gates at PSUM partitions 32d:32d+4 via PE col-group
    tile_position=(0, 32d): d0/d1 matmuls overlap in the array, and one
    ACT/DVE instruction covers both directions' elementwise work
    (rows 4:32 are dead lanes — lane-parallel engines charge free-dim only)
  - h accumulators (A tiles) hold time-ordered contiguous regions
    [fwd 0:4S | bwd 4S:8S] so every matmul AP is 1-D; time reversal is done
    by compile-time block indexing (bwd injection reads block S-1-s, tail
    copies write bwd h to its time slot)
  - c-state in SBUF; tanh(o) off the critical chain; split gate-tanh per
    PSUM bank so it starts as soon as bank0 closes; one STT of the B-term
    runs on GpSimd to unload DVE
  - gate math (tanh-only trick, doubled states):
      sigma(x) = (tanh(x/2)+1)/2 ; ct = 2c ; ht = 2h
      A = (tf+1)*ct ; B = (ti+1)*tg ; ct' = 0.5*A + B
      to' = tanh(o/2) ; tc = tanh(0.5*ct') ; ht = (to'+1)*tc
"""

import sys

for _p in ("/opt/trn_rl_repo",):
    if _p not in sys.path:
        sys.path.insert(0, _p)

import numpy as np
import ml_dtypes

import concourse.bass as bass
import concourse.mybir as mybir
import concourse.tile as tile
from concourse import bass_utils

F32 = mybir.dt.float32
BF16 = mybir.dt.bfloat16
AF = mybir.ActivationFunctionType
ALU = mybir.AluOpType
BF = ml_dtypes.bfloat16

# Problem constants (hardcoded per contract).
B, S, E, H, V, TAGS = 32, 128, 300, 200, 50002, 17
NCORES = 8
BL = B // NCORES          # 4 sentences per core
SB = S * BL               # 512 = time*batch columns per core
XK = (128, 128, 48)       # X^T partition chunks (300 emb dims + 1 ones + pad)
XROWS = 304
X1K = (128, 72, 128, 72, 1)   # layer-1 input chunks: fwd h(128,72), bwd h(128,72), ones
X1ROWS = 401
UCHUNKS = ((0, 128), (128, 128), (256, 128), (384, 16),
           (400, 128), (528, 128), (656, 128), (784, 16))  # 800 units, edge at 400


def _set_seq(s_len):
    """Dev helper: shrink the sequence length (and SB) for fast simulation."""
    global S, SB
    S = s_len
    SB = S * BL


def _prep_weights(emb_table, lstm_Wih0, lstm_Whh0, lstm_b0,
                  lstm_Wih1, lstm_Whh1, lstm_b1, out_w, out_b):
    """Host-side weight transforms shared by all cores (bf16 outputs)."""
    f32 = np.float32

    # layer0 input weights + bias: rows = 300 emb dims + ones row + pad
    w0 = np.zeros((XROWS, 1600), f32)
    for d in range(2):
        wt = lstm_Wih0[d].T.astype(f32).copy()        # [300, 800]
        wt[:, 0:400] *= 0.5                           # i,f halved
        wt[:, 600:800] *= 0.5                         # o halved too
        b = lstm_b0[d].astype(f32).copy()
        b[0:400] *= 0.5
        b[600:800] *= 0.5
        w0[0:300, d * 800:(d + 1) * 800] = wt
        w0[300, d * 800:(d + 1) * 800] = b

    # layer0 recurrent weights: input is ht=2h -> *0.5 ; i,f further *0.5
    u0 = np.zeros((200, 1600), f32)
    for d in range(2):
        ut = lstm_Whh0[d].T.astype(f32) * 0.5
        ut = ut.copy()
        ut[:, 0:400] *= 0.5
        ut[:, 600:800] *= 0.5
        u0[:, d * 800:(d + 1) * 800] = ut

    # layer1 input weights: input is ht-concat (=2x) -> *0.5 ; i,f *0.5 ; bias plain
    w1 = np.zeros((X1ROWS, 1600), f32)
    for d in range(2):
        wt = lstm_Wih1[d].T.astype(f32) * 0.5         # [400, 800]
        wt = wt.copy()
        wt[:, 0:400] *= 0.5
        wt[:, 600:800] *= 0.5
        b = lstm_b1[d].astype(f32).copy()
        b[0:400] *= 0.5
        b[600:800] *= 0.5
        w1[0:400, d * 800:(d + 1) * 800] = wt
        w1[400, d * 800:(d + 1) * 800] = b

    u1 = np.zeros((200, 1600), f32)
    for d in range(2):
        ut = lstm_Whh1[d].T.astype(f32) * 0.5
        ut = ut.copy()
        ut[:, 0:400] *= 0.5
        ut[:, 600:800] *= 0.5
        u1[:, d * 800:(d + 1) * 800] = ut

    # output head: input is ht-concat -> *0.5 ; bias row plain
    ow = np.zeros((X1ROWS, TAGS), f32)
    ow[0:400, :] = out_w.T.astype(f32) * 0.5
    ow[400, :] = out_b.astype(f32)

    i128 = np.eye(128, dtype=f32)
    return {k: v.astype(BF) for k, v in
            {"w0": w0, "u0": u0, "w1": w1, "u1": u1, "ow": ow,
             "i128": i128}.items()}


def _prep_xt(emb_table, words_shard):
    """[304, SB] X^T for one core: col 4t+b = emb[words[b, t]] (time order).

    row 300 = 1.0 (bias row), rows 301:304 = 0.
    """
    emb = np.asarray(emb_table)
    w = np.asarray(words_shard)                       # [BL, S]
    idx_f = w.T.reshape(-1)                           # n = t*BL + b
    xt = np.zeros((XROWS, SB), np.float32)
    xt[0:300, 0:SB] = emb[idx_f].T.astype(np.float32)
    xt[300, :] = 1.0
    return xt.astype(BF)


# --------------------------------------------------------------------------
# Bass program
# --------------------------------------------------------------------------

def _emit_xw_precompute(nc, tc, psum_pool, w_chunks, rhs_chunks, xw, tag):
    """xw[0:cw, ci*8S + 8t + 4d + b] = sum_k w_chunks[k][:, d*800+c0+..].T @ rhs

    rhs_chunks: list of rhs APs [Kp, SB], columns in time order 4t+b (shared
    by both directions).  Output copied PSUM->SBUF (bf16) into 8-wide
    time-ordered interleaved blocks; the injection matmuls pick block
    t=s (fwd) / t=S-1-s (bwd) at compile time.
    """
    nk = len(w_chunks)
    xwr = xw.rearrange("p (c s q b) -> p c s q b", c=8, s=S, q=2, b=BL)
    for d in range(2):
        for ci, (c0, cw) in enumerate(UCHUNKS):
            ps = psum_pool.tile([128, SB], F32, tag=tag, name=f"{tag}_ps")
            col = d * 800 + c0
            for k in range(nk):
                nc.tensor.matmul(
                    ps[0:cw, 0:SB],
                    w_chunks[k][:, col:col + cw],
                    rhs_chunks[k],
                    start=(k == 0), stop=(k == nk - 1),
                )
            src = ps[0:cw, 0:SB].rearrange("p (s b) -> p s b", s=S)
            dst = xwr[0:cw, ci, :, d, :]
            if (d * 8 + ci) % 2 == 0:
                nc.vector.tensor_copy(dst, src)
            else:
                nc.scalar.copy(dst, src)


def _emit_lstm_layer(nc, tc, xw, ua, ub, Aa, Ab, G, TT, TT2, Gw, Cst,
                     Ttile, Atmp, Btmp, TCa, Hh, Hsa, Hsb, I, s_len=None):
    """Emit one full BiLSTM layer (both directions, S steps unrolled).

    Direction d's gates live at PSUM partitions 32d:32d+4 (PE col-groups via
    tile_position=(0, 32d)).  G[i][bank] are per-bank ping-pong PSUM tiles so
    the gate tanh of bank0 starts as soon as bank0's matmuls close.
    Aa: [128, 8*S] bf16, Ab: [72, 8*S] bf16 — h accumulators, two
    time-ordered regions [fwd 0:4S | bwd 4S:8S].
    Gw is a scratch PSUM tile for PE warm-keeper matmuls (HAM K=8/8).
    """
    if s_len is None:
        s_len = S
    FS = 4 * s_len            # bwd region column offset in A tiles
    xwr = xw.rearrange("p (c s q) -> p c s q", c=8, s=s_len)
    # stale-PSUM guard: garbage rows 4:32 flow through tanh and the identity
    # transpose; Inf/NaN there would contaminate via 0*NaN in the PE sum.
    for i in range(2):
        nc.vector.memset(G[i][0][0:36, :], 0.0)
        nc.vector.memset(G[i][1][0:36, :], 0.0)
    nc.vector.memset(Cst[0:36, 0:H], 0.0)

    def inj(g, bank, s):
        for ci, (c0, cw) in enumerate(UCHUNKS):
            if (c0 < 400) != (bank == 0):
                continue
            for d in range(2):
                t = s if d == 0 else s_len - 1 - s
                nc.tensor.matmul(
                    g[32 * d:32 * d + 4, c0 - 400 * bank:c0 - 400 * bank + cw],
                    xwr[0:cw, ci, t, 4 * d:4 * d + 4],
                    I[0:cw, 0:cw],
                    start=(ci == 0 or ci == 4),
                    stop=(s == 0 and (ci == 3 or ci == 7)),
                    skip_group_check=True,
                    tile_position=(0, 32 * d),
                )

    def rec(g, bank, s):
        n0, n1 = (0, 400) if bank == 0 else (400, 800)
        for cj, (Ht, rows) in enumerate(((Hsa, 128), (Hsb, 72))):
            u = ua if rows == 128 else ub
            for d in range(2):
                # prev h of dir d from the step-indexed ring (written step s-1)
                nc.tensor.matmul(
                    g[32 * d:32 * d + 4, n0 - 400 * bank:n1 - 400 * bank],
                    Ht[(s - 1) % 2][0:rows, 4 * d:4 * d + 4],
                    u[0:rows, 800 * d + n0:800 * d + n1],
                    start=False, stop=(cj == 1),
                    skip_group_check=True,
                    tile_position=(0, 32 * d),
                )

    for s in range(s_len):
        g0, g1 = G[s % 2]
        Tt = Ttile[s % 2]
        TC = TCa[s % 2]
        Hc = Hh[s % 2]
        inj(g0, 0, s)
        if s > 0:
            rec(g0, 0, s)
        inj(g1, 1, s)
        if s > 0:
            rec(g1, 1, s)
        # ---- gate activations + cell update (rows 4:32 are dead lanes)
        nc.scalar.activation(Tt[0:36, 0:400], g0[0:36, 0:400], AF.Tanh)
        nc.scalar.activation(Tt[0:36, 400:600], g1[0:36, 0:200], AF.Tanh)
        nc.scalar.activation(Tt[0:36, 600:800], g1[0:36, 200:400], AF.Tanh)
        nc.vector.scalar_tensor_tensor(
            Atmp[0:36, 0:H], Tt[0:36, 200:400], 1.0,
            Cst[0:36, 0:H], ALU.add, ALU.mult)
        nc.vector.scalar_tensor_tensor(
            Btmp[0:36, 0:H], Tt[0:36, 0:200], 1.0,
            Tt[0:36, 400:600], ALU.add, ALU.mult)
        nc.vector.scalar_tensor_tensor(
            Cst[0:36, 0:H], Atmp[0:36, 0:H], 0.5,
            Btmp[0:36, 0:H], ALU.mult, ALU.add)
        nc.scalar.activation(TC[0:36, 0:H], Cst[0:36, 0:H], AF.Tanh,
                             scale=0.5)
        # ---- ht = (to'+1)*tc, then transpose into the A buffers
        nc.vector.scalar_tensor_tensor(
            Hc[0:36, 0:H], Tt[0:36, 600:800], 1.0, TC[0:36, 0:H],
            ALU.add, ALU.mult)
        nc.tensor.matmul(TT[0:128, 0:36], Hc[0:36, 0:128], I[0:36, 0:36],
                         start=True, stop=True, skip_group_check=True)
        nc.tensor.matmul(TT2[0:72, 0:36], Hc[0:36, 128:200], I[0:36, 0:36],
                         start=True, stop=True, skip_group_check=True)
        # step-indexed h ring feeds the next step's recurrent matmuls
        ttr = TT.rearrange("p (j b) -> p j b", b=4)
        tt2r = TT2.rearrange("p (j b) -> p j b", b=4)
        nc.vector.tensor_copy(
            Hsa[s % 2][0:128, 0:8].rearrange("p (j b) -> p j b", b=4),
            ttr[0:128, 0::8, :])
        nc.vector.tensor_copy(
            Hsb[s % 2][0:72, 0:8].rearrange("p (j b) -> p j b", b=4),
            tt2r[0:72, 0::8, :])
        # fwd h_s -> time block s; bwd h (time s_len-1-s) -> its time block
        tb = FS + 4 * (s_len - 1 - s)
        nc.vector.tensor_copy(Aa[0:128, 4 * s:4 * s + 4], TT[0:128, 0:4])
        nc.vector.tensor_copy(Aa[0:128, tb:tb + 4], TT[0:128, 32:36])
        nc.scalar.copy(Ab[0:72, 4 * s:4 * s + 4], TT2[0:72, 0:4])
        nc.scalar.copy(Ab[0:72, tb:tb + 4], TT2[0:72, 32:36])


def _fix_pe_multiwaits(nc):
    """Walrus codegen rejects PE Matmult with >1 sync wait (LDWEIGHTS struct
    has a single wait slot).  Hoist extra waits onto PE NoOps inserted just
    before the offending matmult."""
    total = 0
    for fnc in nc.m.functions:
        for blk in fnc.blocks:
            lst = blk.instructions
            out = []
            for ins in lst:
                si = ins.sync_info
                cap = 1
                if si is not None and len(si.on_wait) > cap:
                    si_cls = type(si)
                    extra = list(si.on_wait[:-cap])
                    keep = si.on_wait[-cap]
                    for j, w in enumerate(extra):
                        nop = mybir.InstNoOp(
                            name=f"{ins.name}_wnop{j}", ins=[], outs=[])
                        nop.engine = ins.engine
                        nop.sync_info = si_cls(on_wait=[w], on_update=[])
                        out.append(nop)
                    ins.sync_info = si_cls(on_wait=[keep],
                                           on_update=list(si.on_update))
                    total += 1
                out.append(ins)
            blk.instructions = out
    return total


def build_program(fix_multiwait=True):
    nc = bass.Bass("TRN2", target_bir_lowering=False, debug=False)

    # ---- DRAM tensors (per-core inputs; SPMD same program)
    d_xt = nc.dram_tensor("xt", [XROWS, SB], BF16, kind="ExternalInput").ap()
    d_w0 = nc.dram_tensor("w0", [XROWS, 1600], BF16, kind="ExternalInput").ap()
    d_u0 = nc.dram_tensor("u0", [200, 1600], BF16, kind="ExternalInput").ap()
    d_w1 = nc.dram_tensor("w1", [X1ROWS, 1600], BF16, kind="ExternalInput").ap()
    d_u1 = nc.dram_tensor("u1", [200, 1600], BF16, kind="ExternalInput").ap()
    d_ow = nc.dram_tensor("ow", [X1ROWS, TAGS], BF16, kind="ExternalInput").ap()
    d_i128 = nc.dram_tensor("i128", [128, 128], BF16, kind="ExternalInput").ap()
    d_out = nc.dram_tensor("out", [BL, S, TAGS], F32, kind="ExternalOutput").ap()

    with tile.TileContext(nc) as tc:
        with tc.sbuf_pool(name="persist", bufs=1) as SP, \
             tc.psum_pool(name="gates", bufs=1) as GP:
            # persistent SBUF tiles
            I = SP.tile([128, 128], BF16, name="ident")
            u0a = SP.tile([128, 1600], BF16, name="u0a")
            u0b = SP.tile([72, 1600], BF16, name="u0b")
            u1a = SP.tile([128, 1600], BF16, name="u1a")
            u1b = SP.tile([72, 1600], BF16, name="u1b")
            xw0 = SP.tile([128, 8 * 8 * S], BF16, name="xw0")
            xw1 = SP.tile([128, 8 * 8 * S], BF16, name="xw1")
            A0a = SP.tile([128, 8 * S], BF16, name="A0a")
            A0b = SP.tile([72, 8 * S], BF16, name="A0b")
            A1a = SP.tile([128, 8 * S], BF16, name="A1a")
            A1b = SP.tile([72, 8 * S], BF16, name="A1b")
            ones = SP.tile([1, SB], BF16, name="ones")
            owc = [SP.tile([kk, TAGS], BF16, name=f"owc{k}")
                   for k, kk in enumerate(X1K)]
            # work tiles (shared by both layers)
            Cst = SP.tile([36, 256], BF16, name="Cst")
            Ttile = [SP.tile([36, 800], BF16, name=f"Ttile{i}") for i in range(2)]
            Atmp = SP.tile([36, 200], BF16, name="Atmp")
            Btmp = SP.tile([36, 200], BF16, name="Btmp")
            TCa = [SP.tile([36, 200], BF16, name=f"TCa{i}") for i in range(2)]
            Hh = [SP.tile([36, 200], BF16, name=f"Hh{i}") for i in range(2)]
            Hsa = [SP.tile([128, 8], BF16, name=f"Hsa{i}") for i in range(2)]
            Hsb = [SP.tile([72, 8], BF16, name=f"Hsb{i}") for i in range(2)]
            # PSUM: per-bank gate ping-pong + transpose + warm-keeper tiles
            G = [[GP.tile([36, 512], F32, name=f"G{i}b{b}") for b in range(2)]
                 for i in range(2)]
            TT = GP.tile([128, 36], F32, name="TT")
            TT2 = GP.tile([72, 36], F32, name="TT2")
            Gw = None

            # ---- load persistent weights
            nc.sync.dma_start(I, d_i128)
            nc.sync.dma_start(u0a, d_u0[0:128, :])
            nc.sync.dma_start(u0b, d_u0[128:200, :])
            nc.sync.dma_start(u1a, d_u1[0:128, :])
            nc.sync.dma_start(u1b, d_u1[128:200, :])
            nc.vector.memset(ones[0:1, 0:SB], 1.0)
            row = 0
            for k, kk in enumerate(X1K):
                nc.sync.dma_start(owc[k], d_ow[row:row + kk, :])
                row += kk

            # ---- phase 2: xw0 precompute
            with tc.sbuf_pool(name="ph2", bufs=1) as P2S, \
                 tc.psum_pool(name="ph2p", bufs=2) as P2P:
                xTc = [P2S.tile([XK[k], SB], BF16, name=f"xTc{k}")
                       for k in range(3)]
                w0c = [P2S.tile([XK[k], 1600], BF16, name=f"w0c{k}")
                       for k in range(3)]
                row = 0
                for k, kk in enumerate(XK):
                    nc.sync.dma_start(xTc[k], d_xt[row:row + kk, :])
                    nc.sync.dma_start(w0c[k], d_w0[row:row + kk, :])
                    row += kk
                _emit_xw_precompute(
                    nc, tc, P2P, w0c,
                    [xTc[k][:, 0:SB] for k in range(3)],
                    xw0, tag="xw0p")

            # ---- phase 3: layer-0 recurrence
            _emit_lstm_layer(nc, tc, xw0, u0a, u0b, A0a, A0b, G, TT, TT2,
                             Gw, Cst, Ttile, Atmp, Btmp, TCa, Hh, Hsa, Hsb, I)

            # ---- phase 4: xw1 precompute (input = A0 buffers + ones);
            # A regions are time-ordered, so rhs is shared by both dirs.
            rhs1 = [A0a[0:128, 0:4 * S], A0b[0:72, 0:4 * S],
                    A0a[0:128, 4 * S:8 * S], A0b[0:72, 4 * S:8 * S],
                    ones[0:1, 0:SB]]

            with tc.sbuf_pool(name="ph4", bufs=1) as P4S, \
                 tc.psum_pool(name="ph4p", bufs=2) as P4P:
                w1c = [P4S.tile([X1K[k], 1600], BF16, name=f"w1c{k}")
                       for k in range(5)]
                row = 0
                for k, kk in enumerate(X1K):
                    nc.sync.dma_start(w1c[k], d_w1[row:row + kk, :])
                    row += kk
                _emit_xw_precompute(nc, tc, P4P, w1c, rhs1, xw1, tag="xw1p")

            # ---- phase 5: layer-1 recurrence
            _emit_lstm_layer(nc, tc, xw1, u1a, u1b, A1a, A1b, G, TT, TT2,
                             Gw, Cst, Ttile, Atmp, Btmp, TCa, Hh, Hsa, Hsb, I)

            # ---- phase 6: output head (A1 regions are time-ordered)
            with tc.sbuf_pool(name="fin", bufs=2) as FS, \
                 tc.psum_pool(name="finp", bufs=2) as FP:
                out_r = d_out.rearrange("b t e -> t b e")
                ts = min(32, S)            # time steps per M-group
                mt = ts * BL
                for m in range(S // ts):
                    t0, t1 = m * ts, (m + 1) * ts
                    po = FP.tile([mt, TAGS], F32, tag="po", name="po")
                    lhs_chunks = [
                        A1a[0:128, 4 * t0:4 * t1],
                        A1b[0:72, 4 * t0:4 * t1],
                        A1a[0:128, 4 * S + 4 * t0:4 * S + 4 * t1],
                        A1b[0:72, 4 * S + 4 * t0:4 * S + 4 * t1],
                        ones[0:1, 0:mt],
                    ]
                    for k in range(5):
                        nc.tensor.matmul(
                            po[0:mt, 0:TAGS], lhs_chunks[k], owc[k],
                            start=(k == 0), stop=(k == 4),
                        )
                    so = FS.tile([mt, TAGS], F32, tag="so", name="so")
                    nc.scalar.activation(so[0:mt, 0:TAGS], po[0:mt, 0:TAGS],
                                         AF.Sigmoid)
                    nc.sync.dma_start(out_r[ts * m:ts * (m + 1), :, :],
                                      so[0:mt, 0:TAGS])

    if fix_multiwait:
        _fix_pe_multiwaits(nc)
    return nc


_CACHE = {}


def kernel(**inputs):
    inputs = {k: np.asarray(v) for k, v in inputs.items()}
    words = inputs["words"]

    shared = _prep_weights(
        inputs["emb_table"], inputs["lstm_Wih0"], inputs["lstm_Whh0"],
        inputs["lstm_b0"], inputs["lstm_Wih1"], inputs["lstm_Whh1"],
        inputs["lstm_b1"], inputs["out_w"], inputs["out_b"])

    in_maps = []
    for c in range(NCORES):
        xt = _prep_xt(inputs["emb_table"], words[c * BL:(c + 1) * BL])
        in_maps.append({"xt": xt, **shared})

    if "nc" not in _CACHE:
        _CACHE["nc"] = build_program()
    nc = _CACHE["nc"]

    res = bass_utils.run_bass_kernel_spmd(
        nc, in_maps, core_ids=list(range(NCORES)),
        trace=_CACHE.get("trace", False),
        tmpdir=_CACHE.get("tmpdir"))
    _CACHE["last_exec_ns"] = res.exec_time_ns
    _CACHE["last_res"] = res

    out = np.concatenate([res.results[c]["out"] for c in range(NCORES)], axis=0)
    return out.astype(np.float32)
